# revision 15
# baseline (speedup 1.0000x reference)
"""Trainium2 Bass kernel for nn_PoseOpt (pose MLP + SMPL forward kinematics).

kernel(**inputs) takes the FULL inputs of reference.setup_inputs() and returns
the FULL (kp, skts, rvecs) tuple. The unique-pose batch (N_unique=16384) is
sharded across 8 NeuronCores (2048 poses each); the small MLP weights /
embedding gathers / rest-pose tree are prepared host-side and replicated.
Each core computes its 2048 unique poses and writes its full 16384-sample
output slice (including the x8 sample repeat) via DMA descriptors.

Device layout: poses are pose-major (partition p, chunk c), u = c*128 + p.
Per-(joint, component) data lives in 16-wide column blocks (col = j*16 + c).
The kinematic chain runs on quaternions: per tree level, one broadcasted
tensor_tensor against a packed sign-permuted local-quat tile and one grouped
tensor_reduce produce all four world-quat components. World matrices are kept
component-major (off-diagonals stored at half scale; the x2 is folded into
activation-copy scales and scalar_tensor_tensor), translation deltas and the
tree prefix-sum are batched, and skts/kp/rvecs are assembled in SBUF and
written out with replicate-by-8 DMA descriptors. GpSimd is avoided entirely
(it contends with DVE for an SBUF port).
"""

import os
import sys

sys.path.insert(0, "/opt/pypackages")
sys.path.insert(0, "/opt/trn_rl_repo")

import numpy as np

J = 24
PARENTS = [-1, 0, 0, 0, 1, 2, 3, 4, 5, 6, 7, 8, 9, 9, 9, 12, 13, 14, 16, 17, 18, 19, 20, 21]
RES_SCALE = 0.1
N = 131072
NU = 16384
SKIP = N // NU
NCORES = 8
U = NU // NCORES          # 2048 poses per core
P = 128
C = U // P                # 16 chunks
E = 16
DIN = J * 3 + E           # 88
W = 128
DOUT = J * 3 + 3          # 75
JC = J * C                # 384

LEVELS = [(1, 4), (4, 7), (7, 10), (10, 15), (15, 18), (18, 20), (20, 22), (22, 24)]
PRUNS = {
    (1, 4): [(1, 4, 0, True)],
    (4, 7): [(4, 7, 1, False)],
    (7, 10): [(7, 10, 4, False)],
    (10, 15): [(10, 13, 7, False), (13, 15, 9, True)],
    (15, 18): [(15, 18, 12, False)],
    (18, 20): [(18, 20, 16, False)],
    (20, 22): [(20, 22, 18, False)],
    (22, 24): [(22, 24, 20, False)],
}
TREE_RUNS = [(1, 4, 0, True), (4, 13, 1, False), (13, 15, 9, True),
             (15, 18, 12, False), (18, 24, 16, False)]

# joint groups for post-chain pipelining (skts DMA rows: 12*64B = 768B)
GROUPS = [(0, 12), (12, 24)]
G_TREE = {
    (0, 12): [(1, 4, 0, True), (4, 12, 1, False)],
    (12, 24): [(12, 13, 9, False), (13, 15, 9, True), (15, 18, 12, False),
               (18, 24, 16, False)],
}
G_PREFIX = {
    (0, 12): [(1, 4, 0, True), (4, 7, 1, False), (7, 10, 4, False),
              (10, 12, 7, False)],
    (12, 24): [(12, 13, 9, False), (13, 15, 9, True), (15, 18, 12, False),
               (18, 20, 16, False), (20, 22, 18, False), (22, 24, 20, False)],
}
# chain levels available before each group can run
G_CHAIN = {(0, 12): LEVELS[:4], (12, 24): LEVELS[4:]}
GLAST = GROUPS[-1]
# DMA ring per group: first group on sync, later groups on the ACT HWDGE ring
G_DMA_ENG = {(0, 12): "sync", (12, 24): "scalar"}

# packed local-quat variant spec: LQVC[..., v*4+i] = sign * local_q[src]
# world_q[v] = dot(parent_q, variant_v)
VAR_SPEC = [
    [(0, 1.0), (1, -1.0), (2, -1.0), (3, -1.0)],   # w_out
    [(1, 1.0), (0, 1.0), (3, 1.0), (2, -1.0)],     # x_out
    [(2, 1.0), (3, -1.0), (0, 1.0), (1, 1.0)],     # y_out
    [(3, 1.0), (2, 1.0), (1, -1.0), (0, 1.0)],     # z_out
]

_CACHE = {}


def _build(debug_taps=False):
    import concourse.bacc as bacc
    import concourse.mybir as mybir
    import concourse.tile as tile

    F32 = mybir.dt.float32
    F32R = mybir.dt.float32r
    AF = mybir.ActivationFunctionType
    ALU = mybir.AluOpType
    AX = mybir.AxisListType
    use_f32r = os.environ.get("KERNEL_F32R", "0") == "1"

    def mmcast(ap):
        return ap.bitcast(F32R) if use_f32r else ap

    nc = bacc.Bacc("TRN2", target_bir_lowering=False, debug=False, num_devices=NCORES)

    # packed small-constant input: one DMA instead of ten
    # cols: w0[0:128] w1[128:256] w2[256:384] w3p[384:459] i75[459:534]
    #       b0 534 b1 535 b2 536 tl1c[537:1689] tl2c[1689:2841]
    PXC = 2841
    xT_i = nc.declare_dram_parameter("xT", [DIN, U], F32, isOutput=False)
    st_i = nc.declare_dram_parameter("stg0", [P, DOUT * C], F32, isOutput=False)
    pk_i = nc.declare_dram_parameter("pack", [P, PXC], F32, isOutput=False)

    dbg = {}
    if debug_taps:
        for nm, sz in [("lqvc", JC * 16), ("wqi", JC * 4), ("wrc", JC * 9),
                       ("dc", JC * 3), ("wtc", JC * 3), ("tii", JC * 3)]:
            dbg[nm] = nc.declare_dram_parameter("dbg_" + nm, [P, sz], F32, isOutput=True)

    kp_o = nc.declare_dram_parameter("kp_o", [U * SKIP, J, 3], F32, isOutput=True)
    sk_o = nc.declare_dram_parameter("sk_o", [U * SKIP, J, 4, 4], F32, isOutput=True)
    rv_o = nc.declare_dram_parameter("rv_o", [U * SKIP, J, 3], F32, isOutput=True)

    with tile.TileContext(nc) as tc:
        with (
            tc.tile_pool(name="w", bufs=1) as wpool,
            tc.tile_pool(name="big", bufs=1) as big,
            tc.tile_pool(name="psum", bufs=4, space="PSUM") as psum,
            tc.tile_pool(name="psum4", bufs=4, space="PSUM") as psum4,
        ):
            # ---------------- inputs ----------------
            PACK = wpool.tile([P, PXC], F32)
            xT = big.tile([DIN, U], F32, tag="g_xt")       # slot reused by WQC
            nc.sync.dma_start(xT[:, 0:512], xT_i[:, 0:512])
            nc.scalar.dma_start(PACK[:, 0:537], pk_i[:, 0:537])
            nc.sync.dma_start(xT[:, 512:], xT_i[:, 512:])
            nc.scalar.dma_start(PACK[:, 537:], pk_i[:, 537:])
            STG = big.tile([P, DOUT * C], F32, tag="g_stg")
            nc.scalar.dma_start(STG[:], st_i[:])
            w0 = PACK[0:DIN, 0:128]
            w1 = PACK[0:W, 128:256]
            w2 = PACK[0:W, 256:384]
            w3 = PACK[0:W, 384:459]
            b0 = PACK[0:W, 534:535]
            b1 = PACK[0:W, 535:536]
            b2 = PACK[0:W, 536:537]
            TL1 = PACK[:, 537:537 + 3 * JC]
            TL2 = PACK[:, 537 + 3 * JC:537 + 6 * JC]
            halfpi = wpool.tile([P, 1], F32)
            nc.vector.memset(halfpi[:], float(np.pi / 2))

            # skts constant row [0,0,0,1] written while engines are idle
            SK = big.tile([P, C * J * 16], F32, tag="g_skt")
            skv = SK.rearrange("p (c j e) -> p c j e", j=J, e=16)
            nc.vector.memset(skv[:, :, :, 12:15], 0.0)
            nc.vector.memset(skv[:, :, :, 15], 1.0)

            # ---------------- MLP ----------------
            NCHUNK = 512
            h1 = big.tile([W, U], F32, tag="g_h1")         # reused by RV2
            h2 = big.tile([W, U], F32, tag="g_h2")         # reused by PRC
            h3 = big.tile([W, U], F32, tag="g_h3")         # reused by WRC
            with nc.named_scope("mlp"):
                for m in range(U // NCHUNK):
                    sl = slice(m * NCHUNK, (m + 1) * NCHUNK)
                    ps = psum.tile([W, NCHUNK], F32, tag="mm")
                    nc.tensor.matmul(ps[:], mmcast(w0), mmcast(xT[:, sl]), start=True, stop=True)
                    nc.scalar.activation(h1[:, sl], ps[:], AF.Relu, bias=b0)
                for m in range(U // NCHUNK):
                    sl = slice(m * NCHUNK, (m + 1) * NCHUNK)
                    ps = psum.tile([W, NCHUNK], F32, tag="mm")
                    nc.tensor.matmul(ps[:], mmcast(w1), mmcast(h1[:, sl]), start=True, stop=True)
                    nc.scalar.activation(h2[:, sl], ps[:], AF.Relu, bias=b1)
                for m in range(U // NCHUNK):
                    sl = slice(m * NCHUNK, (m + 1) * NCHUNK)
                    ps = psum.tile([W, NCHUNK], F32, tag="mm")
                    nc.tensor.matmul(ps[:], mmcast(w2), mmcast(h2[:, sl]), start=True, stop=True)
                    nc.scalar.activation(h3[:, sl], ps[:], AF.Relu, bias=b2)
                # layer 4; residual-plus-base via fused STT (STG preloaded with base)
                for c in range(C):
                    ps = psum4.tile([P, DOUT], F32, tag="mm4")
                    nc.tensor.matmul(ps[:], mmcast(h3[:, c * P:(c + 1) * P]), mmcast(w3),
                                     start=True, stop=True)
                    sv = STG.rearrange("p (q c) -> p q c", c=C)[:, :, c]
                    nc.vector.scalar_tensor_tensor(sv, ps[:], 1.0, sv, ALU.mult, ALU.add)

            rx = STG[:, 0 * JC:1 * JC]
            ry = STG[:, 1 * JC:2 * JC]
            rz = STG[:, 2 * JC:3 * JC]

            # rvecs output depends only on STG: assemble + DMA it first
            RV2 = big.tile([P, C * 2 * J * 3], F32, tag="g_rv2")
            rvv = RV2.rearrange("p (c d j k) -> p c d j k", d=2, j=J, k=3)
            rv_cja = STG.rearrange("p (q c) -> p q c", c=C)[:, 0:J * 3, :] \
                        .rearrange("p (a j) c -> p c j a", a=3)
            with nc.named_scope("rv_out"):
                for d in range(2):
                    nc.scalar.copy(rvv[:, :, d], rv_cja)
                rv_flat = rv_o.rearrange("(c p r d) j k -> p c r (d j k)", c=C, p=P,
                                         r=SKIP // 2, d=2)
                rv_src = RV2.rearrange("p (c e) -> p c e", c=C)
                for r in range(SKIP // 2):
                    nc.sync.dma_start(rv_flat[:, :, r], rv_src[:])

            # ---------------- Rodrigues -> packed local-quat variants ----------------
            LQVC = big.tile([P, JC * 16], F32, tag="g_lqvc")
            lqv16 = LQVC.rearrange("p (g e) -> p g e", e=16)
            with nc.named_scope("rodrigues"):
                sqx = big.tile([P, JC], F32, tag="t0")
                sqy = big.tile([P, JC], F32, tag="t1")
                sqz = big.tile([P, JC], F32, tag="t2")
                nc.scalar.activation(sqx[:], rx, AF.Square)
                nc.scalar.activation(sqy[:], ry, AF.Square)
                nc.scalar.activation(sqz[:], rz, AF.Square)
                t2_ = big.tile([P, JC], F32, tag="t3")
                nc.vector.tensor_tensor(t2_[:], sqx[:], sqy[:], ALU.add)
                nc.vector.tensor_tensor(t2_[:], t2_[:], sqz[:], ALU.add)
                th = big.tile([P, JC], F32, tag="t4")
                nc.scalar.activation(th[:], t2_[:], AF.Sqrt)
                thc = big.tile([P, JC], F32, tag="t5")
                nc.vector.tensor_scalar_max(thc[:], th[:], 1e-8)
                inv = big.tile([P, JC], F32, tag="t6")
                scratch = big.tile([P, JC], F32, tag="t7")
                nc.vector.reciprocal_approx_accurate(inv[:], thc[:], scratch[:])
                sh = big.tile([P, JC], F32, tag="t8")
                nc.scalar.activation(sh[:], th[:], AF.Sin, scale=0.5)
                sv_ = big.tile([P, JC], F32, tag="t9")
                nc.vector.tensor_tensor(sv_[:], sh[:], inv[:], ALU.mult)
                nsv = big.tile([P, JC], F32, tag="t10")
                nc.vector.tensor_scalar_mul(nsv[:], sv_[:], -1.0)
                for v in range(4):
                    nc.scalar.activation(lqv16[:, :, v * 4 + v], th[:], AF.Sin,
                                         scale=0.5, bias=halfpi[:])
                rblk = {1: rx, 2: ry, 3: rz}
                for v, spec in enumerate(VAR_SPEC):
                    for pos, (src_, sgn) in enumerate(spec):
                        if src_ == 0:
                            continue
                        s_op = sv_ if sgn > 0 else nsv
                        nc.vector.tensor_tensor(lqv16[:, :, v * 4 + pos],
                                                rblk[src_], s_op[:], ALU.mult)

            # ---------------- quaternion chain ----------------
            WQI = big.tile([P, JC * 4], F32, tag="g_wqi")
            wqi4 = WQI.rearrange("p (g i) -> p g i", i=4)
            TMP = big.tile([P, 5 * C * 16], F32, tag="g_tmp")
            REP = big.tile([P, 3 * C * 4], F32, tag="g_rep")

            def chain_levels(levels):
                # per parent-run: TT (parent quat broadcast over variants) + reduce
                for lev in levels:
                    for (clo, chi, plo, bc) in PRUNS[lev]:
                        w_ = chi - clo
                        if bc:
                            # replicate the single parent block w_ times first
                            rsrc = WQI[:, plo * C * 4:(plo + 1) * C * 4] \
                                .rearrange("p (j g) -> p j g", j=1) \
                                .broadcast_to([P, w_, C * 4])
                            rep = REP[:, 0:w_ * C * 4]
                            nc.vector.tensor_copy(rep.rearrange("p (j g) -> p j g", g=C * 4), rsrc)
                            par = rep
                        else:
                            par = WQI[:, plo * C * 4:(plo + w_) * C * 4]
                        a_in = par.rearrange("p (g v i) -> p g v i", v=1, i=4) \
                                  .broadcast_to([P, w_ * C, 4, 4])
                        b_in = LQVC[:, clo * C * 16:chi * C * 16] \
                            .rearrange("p (g v i) -> p g v i", v=4, i=4)
                        tview = TMP[:, 0:w_ * C * 16].rearrange("p (g v i) -> p g v i", v=4, i=4)
                        nc.vector.tensor_tensor(tview, a_in, b_in, ALU.mult)
                        out = WQI[:, clo * C * 4:chi * C * 4].rearrange("p (g v) -> p g v", v=4)
                        nc.vector.tensor_reduce(out, tview, axis=AX.X, op=ALU.add)

            with nc.named_scope("chain_root"):
                # root world quat = root local quat
                nc.scalar.activation(wqi4[:, 0:C, 0], th[:, 0:C], AF.Sin,
                                     scale=0.5, bias=halfpi[:])
                for i, blk in ((1, rx), (2, ry), (3, rz)):
                    nc.vector.tensor_tensor(wqi4[:, 0:C, i], blk[:, 0:C],
                                            sv_[:, 0:C], ALU.mult)

            # ---------------- per-group post-chain tiles ----------------
            WQC = big.tile([P, JC * 4], F32, tag="g_xt")
            WRC = big.tile([P, JC * 9], F32, tag="g_h3")
            wrb = [WRC[:, m * JC:(m + 1) * JC] for m in range(9)]
            PRC = big.tile([P, JC * 9], F32, tag="g_h2")
            dC = big.tile([P, JC * 3], F32, tag="g_dc")
            WTC = big.tile([P, JC * 3], F32, tag="g_wtc")
            TII = big.tile([P, JC * 3], F32, tag="g_tii")
            KP2 = big.tile([P, C * 2 * J * 3], F32, tag="g_bt")
            kpv = KP2.rearrange("p (c d j k) -> p c d j k", d=2, j=J, k=3)

            qw = WQC[:, 0 * JC:1 * JC]
            qx = WQC[:, 1 * JC:2 * JC]
            qy = WQC[:, 2 * JC:3 * JC]
            qz = WQC[:, 3 * JC:4 * JC]
            wr_mjc = WRC.rearrange("p (m j c) -> p m j c", m=9, j=J)
            pr_mjc = PRC.rearrange("p (m j c) -> p m j c", m=9, j=J)
            prb = [PRC[:, m * JC:(m + 1) * JC] for m in range(9)]
            tl1b = [TL1[:, k * JC:(k + 1) * JC] for k in range(3)]
            tl2b = [TL2[:, k * JC:(k + 1) * JC] for k in range(3)]
            wt_kjc = WTC.rearrange("p (k j c) -> p k j c", k=3, j=J)
            dc_kjc = dC.rearrange("p (k j c) -> p k j c", k=3, j=J)
            wtb = [WTC[:, k * JC:(k + 1) * JC] for k in range(3)]
            wr_cj = WRC.rearrange("p (m j c) -> p m c j", m=9, j=J)
            ti_cj = TII.rearrange("p (r j c) -> p r c j", r=3, j=J)
            wt_cjk = WTC.rearrange("p (k j c) -> p c j k", k=3, j=J)
            sk_flat = sk_o.rearrange("(c p r) j a b -> p c r j (a b)", c=C, p=P, r=SKIP)
            sk_srcj = SK.rearrange("p (c j e) -> p c j e", j=J, e=16)

            def sl_(blk, lo, hi):
                # jc-slice [lo*C, hi*C) of a 384-wide comp block view
                return blk[:, lo * C:hi * C]

            def emit_group(g):
                jlo, jhi = g
                S = slice(jlo * C, jhi * C)
                with nc.named_scope(f"deint_{jlo}"):
                    for i in range(4):
                        nc.scalar.copy(WQC[:, i * JC + jlo * C:i * JC + jhi * C],
                                       wqi4[:, S, i])
                with nc.named_scope(f"q2m_{jlo}"):
                    xx = big.tile([P, JC], F32, tag="t0")
                    yy = big.tile([P, JC], F32, tag="t1")
                    zz = big.tile([P, JC], F32, tag="t2")
                    xy = big.tile([P, JC], F32, tag="t3")
                    xz = big.tile([P, JC], F32, tag="t4")
                    yz = big.tile([P, JC], F32, tag="t5")
                    wx = big.tile([P, JC], F32, tag="t6")
                    wy = big.tile([P, JC], F32, tag="t7")
                    wz = big.tile([P, JC], F32, tag="t8")
                    s_ = big.tile([P, JC], F32, tag="t9")
                    for dst_, a_ in ((xx, qx), (yy, qy), (zz, qz)):
                        nc.scalar.activation(sl_(dst_, jlo, jhi), sl_(a_, jlo, jhi), AF.Square)
                    for dst_, a_, b_ in ((xy, qx, qy), (xz, qx, qz), (yz, qy, qz),
                                         (wx, qw, qx), (wy, qw, qy), (wz, qw, qz)):
                        nc.vector.tensor_tensor(sl_(dst_, jlo, jhi), sl_(a_, jlo, jhi),
                                                sl_(b_, jlo, jhi), ALU.mult)
                    for m, p1, p2 in ((0, yy, zz), (4, xx, zz), (8, xx, yy)):
                        nc.vector.tensor_tensor(sl_(s_, jlo, jhi), sl_(p1, jlo, jhi),
                                                sl_(p2, jlo, jhi), ALU.add)
                        nc.scalar.activation(sl_(wrb[m], jlo, jhi), sl_(s_, jlo, jhi),
                                             AF.Copy, scale=-2.0, bias=1.0)
                    for m, p1, p2, op in ((1, xy, wz, ALU.add), (3, xy, wz, ALU.subtract),
                                          (2, xz, wy, ALU.subtract), (6, xz, wy, ALU.add),
                                          (5, yz, wx, ALU.add), (7, yz, wx, ALU.subtract)):
                        nc.vector.tensor_tensor(sl_(wrb[m], jlo, jhi), sl_(p1, jlo, jhi),
                                                sl_(p2, jlo, jhi), op)
                with nc.named_scope(f"deltas_{jlo}"):
                    if jlo == 0:
                        nc.vector.memset(pr_mjc[:, :, 0], 0.0)
                    for (clo, chi, plo, bc) in G_TREE[g]:
                        dst = pr_mjc[:, :, clo:chi]
                        if bc:
                            src = wr_mjc[:, :, plo:plo + 1].broadcast_to([P, 9, chi - clo, C])
                        else:
                            src = wr_mjc[:, :, plo:plo + (chi - clo)]
                        nc.scalar.copy(dst, src)
                    m1 = big.tile([P, JC], F32, tag="t0")
                    m2 = big.tile([P, JC], F32, tag="t1")
                    for a in range(3):
                        acc = sl_(dC[:, a * JC:(a + 1) * JC], jlo, jhi)
                        tls = [tl1b[k] if k == a else tl2b[k] for k in range(3)]
                        nc.vector.tensor_tensor(sl_(m1, jlo, jhi), sl_(prb[0 * 3 + a], jlo, jhi),
                                                sl_(tls[0], jlo, jhi), ALU.mult)
                        nc.vector.tensor_tensor(sl_(m2, jlo, jhi), sl_(prb[1 * 3 + a], jlo, jhi),
                                                sl_(tls[1], jlo, jhi), ALU.mult)
                        nc.vector.tensor_tensor(sl_(m1, jlo, jhi), sl_(m1, jlo, jhi),
                                                sl_(m2, jlo, jhi), ALU.add)
                        nc.vector.tensor_tensor(sl_(m2, jlo, jhi), sl_(prb[2 * 3 + a], jlo, jhi),
                                                sl_(tls[2], jlo, jhi), ALU.mult)
                        nc.vector.tensor_tensor(acc, sl_(m1, jlo, jhi), sl_(m2, jlo, jhi),
                                                ALU.add)
                with nc.named_scope(f"prefix_{jlo}"):
                    if jlo == 0:
                        pel = STG.rearrange("p (q c) -> p q c", c=C)[:, DOUT - 3:DOUT, :]
                        nc.vector.tensor_copy(wt_kjc[:, :, 0], pel)
                    for (clo, chi, plo, bc) in G_PREFIX[g]:
                        dst = wt_kjc[:, :, clo:chi]
                        if bc:
                            src = wt_kjc[:, :, plo:plo + 1].broadcast_to([P, 3, chi - clo, C])
                        else:
                            src = wt_kjc[:, :, plo:plo + (chi - clo)]
                        nc.vector.tensor_tensor(dst, src, dc_kjc[:, :, clo:chi], ALU.add)
                if jlo == GLAST[0]:
                    with nc.named_scope("kp_out"):
                        for d_ in range(2):
                            nc.scalar.copy(kpv[:, :, d_], wt_cjk[:])
                        kp_flat = kp_o.rearrange("(c p r d) j k -> p c r (d j k)", c=C,
                                                 p=P, r=SKIP // 2, d=2)
                        kp_src = KP2.rearrange("p (c e) -> p c e", c=C)
                        for r_ in range(SKIP // 2):
                            nc.sync.dma_start(kp_flat[:, :, r_], kp_src[:])
                with nc.named_scope(f"tinv_{jlo}"):
                    n1 = big.tile([P, JC], F32, tag="t2")
                    n2 = big.tile([P, JC], F32, tag="t3")
                    for r in range(3):
                        acc = sl_(TII[:, r * JC:(r + 1) * JC], jlo, jhi)

                        def term(dst_, k, r=r):
                            if k == r:
                                nc.vector.tensor_tensor(dst_, sl_(wrb[r * 3 + k], jlo, jhi),
                                                        sl_(wtb[k], jlo, jhi), ALU.mult)
                            else:
                                nc.vector.scalar_tensor_tensor(dst_, sl_(wrb[r * 3 + k], jlo, jhi),
                                                               2.0, sl_(wtb[k], jlo, jhi),
                                                               ALU.mult, ALU.mult)
                        term(sl_(n1, jlo, jhi), 0)
                        term(sl_(n2, jlo, jhi), 1)
                        nc.vector.tensor_tensor(sl_(n1, jlo, jhi), sl_(n1, jlo, jhi),
                                                sl_(n2, jlo, jhi), ALU.add)
                        term(sl_(n2, jlo, jhi), 2)
                        nc.vector.tensor_tensor(acc, sl_(n1, jlo, jhi), sl_(n2, jlo, jhi),
                                                ALU.add)
                with nc.named_scope(f"sk_{jlo}"):
                    for r in range(3):
                        for k in range(3):
                            scale = 1.0 if r == k else 2.0
                            if r == 1:
                                nc.vector.tensor_scalar_mul(skv[:, :, jlo:jhi, r * 4 + k],
                                                            wr_cj[:, r * 3 + k, :, jlo:jhi],
                                                            scale)
                            else:
                                nc.scalar.activation(skv[:, :, jlo:jhi, r * 4 + k],
                                                     wr_cj[:, r * 3 + k, :, jlo:jhi],
                                                     AF.Copy, scale=scale)
                        if r == 1:
                            nc.vector.tensor_scalar_mul(skv[:, :, jlo:jhi, r * 4 + 3],
                                                        ti_cj[:, r, :, jlo:jhi], -1.0)
                        else:
                            nc.scalar.activation(skv[:, :, jlo:jhi, r * 4 + 3],
                                                 ti_cj[:, r, :, jlo:jhi], AF.Copy, scale=-1.0)
                with nc.named_scope(f"skdma_{jlo}"):
                    eng = nc.sync if G_DMA_ENG[g] == "sync" else nc.scalar
                    for r in range(SKIP):
                        eng.dma_start(sk_flat[:, :, r, jlo:jhi],
                                      sk_srcj[:, :, jlo:jhi])

            # chain piece -> group, interleaved for earliest first output
            for g in GROUPS:
                with nc.named_scope(f"chain_{g[0]}"):
                    chain_levels(G_CHAIN[g])
                emit_group(g)

            if debug_taps:
                nc.sync.dma_start(dbg["lqvc"][:], LQVC[:])
                nc.sync.dma_start(dbg["wqi"][:], WQI[:])
                nc.sync.dma_start(dbg["wrc"][:], WRC[:])
                nc.sync.dma_start(dbg["dc"][:], dC[:])
                nc.sync.dma_start(dbg["wtc"][:], WTC[:])
                nc.sync.dma_start(dbg["tii"][:], TII[:])

    nc.compile()
    return nc


def _get_nc():
    if "nc" not in _CACHE:
        _CACHE["nc"] = _build()
    return _CACHE["nc"]


def _maybe_enable_trace():
    import types
    try:
        import antenv
        if hasattr(antenv, "axon_hooks"):
            return True
        hooks_mod = types.ModuleType("antenv.axon_hooks")
        _hook = [None]
        hooks_mod.set_axon_ntff_profile_hook = lambda h: _hook.__setitem__(0, h)
        hooks_mod.get_axon_ntff_profile_hook = lambda: _hook[0]
        sys.modules["antenv.axon_hooks"] = hooks_mod
        import antenv as _ae
        _ae.axon_hooks = hooks_mod
        from trn_agent_boot.trn_boot import _ntff_profile_via_ctypes
        hooks_mod.set_axon_ntff_profile_hook(
            _ntff_profile_via_ctypes("/opt/axon/libaxon_pjrt.so"))
        return True
    except Exception:
        return False


def kernel(bones, kp3d, kp_idxs, emb_table, pelvis_buf, rest_pose,
           W0, b0, W1, b1, W2, b2, W3, b3, N_unique):
    from concourse.bass_utils import run_bass_kernel_spmd

    bones = np.asarray(bones, dtype=np.float32)
    emb_table = np.asarray(emb_table, dtype=np.float32)
    pelvis_buf = np.asarray(pelvis_buf, dtype=np.float32)
    rest_pose = np.asarray(rest_pose, dtype=np.float32)
    kp_idxs = np.asarray(kp_idxs)
    W0 = np.asarray(W0, dtype=np.float32); b0 = np.asarray(b0, dtype=np.float32)
    W1 = np.asarray(W1, dtype=np.float32); b1 = np.asarray(b1, dtype=np.float32)
    W2 = np.asarray(W2, dtype=np.float32); b2 = np.asarray(b2, dtype=np.float32)
    W3 = np.asarray(W3, dtype=np.float32); b3 = np.asarray(b3, dtype=np.float32)

    nu = int(N_unique)
    assert nu == NU and bones.shape[0] == N

    rvecs = bones[::SKIP].reshape(nu, J * 3)
    kp_u = np.asarray(kp_idxs[::SKIP], dtype=np.int64)
    embs = emb_table[kp_u]
    pelvis = pelvis_buf[kp_u]

    qperm = np.empty(DOUT, np.int64)
    for q in range(72):
        qperm[q] = (q % J) * 3 + (q // J)
    qperm[72:] = np.arange(72, 75)
    W3p = np.ascontiguousarray(W3[:, qperm] * RES_SCALE)
    b3p = (b3[qperm] * RES_SCALE).astype(np.float32)

    tl = np.zeros((J, 3), np.float32)
    for j in range(1, J):
        tl[j] = rest_pose[j] - rest_pose[PARENTS[j]]
    # comp-major TL rows: [k*JC + j*16 + c] = tl[j, k] (and 2x version)
    tlc = np.repeat(tl.T.reshape(3, J), C, axis=1).reshape(3 * JC)
    pack = np.zeros((P, 2841), np.float32)
    pack[0:DIN, 0:128] = W0
    pack[0:W, 128:256] = W1
    pack[0:W, 256:384] = W2
    pack[0:W, 384:459] = W3p
    pack[0:DOUT, 459:534] = np.eye(DOUT, dtype=np.float32)
    pack[0:W, 534] = b0.ravel()
    pack[0:W, 535] = b1.ravel()
    pack[0:W, 536] = b2.ravel()
    pack[:, 537:537 + 3 * JC] = tlc
    pack[:, 537 + 3 * JC:537 + 6 * JC] = 2.0 * tlc

    in_maps = []
    for core in range(NCORES):
        sl = slice(core * U, (core + 1) * U)
        rv_c = rvecs[sl]
        x = np.concatenate([rv_c, embs[sl]], axis=1)
        xT = np.ascontiguousarray(x.T)
        base = np.empty((U, DOUT), np.float32)
        base[:, :72] = rv_c.reshape(U, J, 3).transpose(0, 2, 1).reshape(U, 72)
        base[:, 72:] = pelvis[sl]
        base += b3p
        # stg0[p, q*16+c] = base[c*128+p, q]
        stg0 = np.ascontiguousarray(
            base.reshape(C, P, DOUT).transpose(1, 2, 0).reshape(P, DOUT * C))
        in_maps.append(dict(xT=xT, stg0=stg0, pack=pack))

    trace = bool(os.environ.get("KERNEL_TRACE")) and _maybe_enable_trace()
    nc = _get_nc()
    res = run_bass_kernel_spmd(nc, in_maps, core_ids=list(range(NCORES)), trace=trace)
    if trace:
        _CACHE["last_exec_ns"] = res.exec_time_ns
        _CACHE["last_trace"] = res.instructions_and_trace[1] if res.instructions_and_trace else None
        _CACHE["last_scope_times"] = res.per_core_scope_times

    _CACHE["last_res"] = res.results
    kp = np.concatenate([r["kp_o"] for r in res.results], axis=0)
    skts = np.concatenate([r["sk_o"] for r in res.results], axis=0)
    rv = np.concatenate([r["rv_o"] for r in res.results], axis=0)
    return kp, skts, rv


# revision 17
# speedup vs baseline: 1.1227x; 1.1227x over previous
"""Trainium2 Bass kernel for nn_PoseOpt (pose MLP + SMPL forward kinematics).

kernel(**inputs) takes the FULL inputs of reference.setup_inputs() and returns
the FULL (kp, skts, rvecs) tuple. The unique-pose batch (N_unique=16384) is
sharded across 8 NeuronCores (2048 poses each); the small MLP weights /
embedding gathers / rest-pose tree are prepared host-side and replicated.
Each core computes its 2048 unique poses and writes its full 16384-sample
output slice (including the x8 sample repeat) via DMA descriptors.

Device layout: poses are pose-major (partition p, chunk c), u = c*128 + p.
Per-(joint, component) data lives in 16-wide column blocks (col = j*16 + c).
The kinematic chain runs on quaternions: per tree level, one broadcasted
tensor_tensor against a packed sign-permuted local-quat tile and one grouped
tensor_reduce produce all four world-quat components. World matrices are kept
component-major (off-diagonals stored at half scale; the x2 is folded into
activation-copy scales and scalar_tensor_tensor), translation deltas and the
tree prefix-sum are batched, and skts/kp/rvecs are assembled in SBUF and
written out with replicate-by-8 DMA descriptors. GpSimd is avoided entirely
(it contends with DVE for an SBUF port).
"""

import os
import sys

sys.path.insert(0, "/opt/pypackages")
sys.path.insert(0, "/opt/trn_rl_repo")

import numpy as np

J = 24
PARENTS = [-1, 0, 0, 0, 1, 2, 3, 4, 5, 6, 7, 8, 9, 9, 9, 12, 13, 14, 16, 17, 18, 19, 20, 21]
RES_SCALE = 0.1
N = 131072
NU = 16384
SKIP = N // NU
NCORES = 8
U = NU // NCORES          # 2048 poses per core
P = 128
C = U // P                # 16 chunks
E = 16
DIN = J * 3 + E           # 88
W = 128
DOUT = J * 3 + 3          # 75
JC = J * C                # 384

LEVELS = [(1, 4), (4, 7), (7, 10), (10, 15), (15, 18), (18, 20), (20, 22), (22, 24)]
PRUNS = {
    (1, 4): [(1, 4, 0, True)],
    (4, 7): [(4, 7, 1, False)],
    (7, 10): [(7, 10, 4, False)],
    (10, 15): [(10, 13, 7, False), (13, 15, 9, True)],
    (15, 18): [(15, 18, 12, False)],
    (18, 20): [(18, 20, 16, False)],
    (20, 22): [(20, 22, 18, False)],
    (22, 24): [(22, 24, 20, False)],
}
TREE_RUNS = [(1, 4, 0, True), (4, 13, 1, False), (13, 15, 9, True),
             (15, 18, 12, False), (18, 24, 16, False)]

# joint groups for post-chain pipelining (skts DMA rows: 12*64B = 768B)
GROUPS = [(0, 12), (12, 24)]
G_TREE = {
    (0, 12): [(1, 4, 0, True), (4, 12, 1, False)],
    (12, 24): [(12, 13, 9, False), (13, 15, 9, True), (15, 18, 12, False),
               (18, 24, 16, False)],
}
G_PREFIX = {
    (0, 12): [(1, 4, 0, True), (4, 7, 1, False), (7, 10, 4, False),
              (10, 12, 7, False)],
    (12, 24): [(12, 13, 9, False), (13, 15, 9, True), (15, 18, 12, False),
               (18, 20, 16, False), (20, 22, 18, False), (22, 24, 20, False)],
}
# chain levels available before each group can run
G_CHAIN = {(0, 12): LEVELS[:4], (12, 24): LEVELS[4:]}
GLAST = GROUPS[-1]
# DMA ring per group: first group on sync, later groups on the ACT HWDGE ring
G_DMA_ENG = {(0, 12): "sync", (12, 24): "sync"}

# packed local-quat variant spec: LQVC[..., v*4+i] = sign * local_q[src]
# world_q[v] = dot(parent_q, variant_v)
VAR_SPEC = [
    [(0, 1.0), (1, -1.0), (2, -1.0), (3, -1.0)],   # w_out
    [(1, 1.0), (0, 1.0), (3, 1.0), (2, -1.0)],     # x_out
    [(2, 1.0), (3, -1.0), (0, 1.0), (1, 1.0)],     # y_out
    [(3, 1.0), (2, 1.0), (1, -1.0), (0, 1.0)],     # z_out
]

_CACHE = {}


def _build(debug_taps=False):
    import concourse.bacc as bacc
    import concourse.mybir as mybir
    import concourse.tile as tile

    F32 = mybir.dt.float32
    F32R = mybir.dt.float32r
    AF = mybir.ActivationFunctionType
    ALU = mybir.AluOpType
    AX = mybir.AxisListType
    use_f32r = os.environ.get("KERNEL_F32R", "0") == "1"

    def mmcast(ap):
        return ap.bitcast(F32R) if use_f32r else ap

    nc = bacc.Bacc("TRN2", target_bir_lowering=False, debug=False, num_devices=NCORES)

    # packed small-constant input: one DMA instead of ten
    # cols: w0[0:128] w1[128:256] w2[256:384] w3p[384:459] i75[459:534]
    #       b0 534 b1 535 b2 536 tl1c[537:1689] tl2c[1689:2841]
    PXC = 2841
    xT_i = nc.declare_dram_parameter("xT", [DIN, U], F32, isOutput=False)
    st_i = nc.declare_dram_parameter("stg0", [P, DOUT * C], F32, isOutput=False)
    pk_i = nc.declare_dram_parameter("pack", [P, PXC], F32, isOutput=False)

    dbg = {}
    if debug_taps:
        for nm, sz in [("lqvc", JC * 16), ("wqi", JC * 4), ("wrc", JC * 9),
                       ("dc", JC * 3), ("wtc", JC * 3), ("tii", JC * 3)]:
            dbg[nm] = nc.declare_dram_parameter("dbg_" + nm, [P, sz], F32, isOutput=True)

    kp_o = nc.declare_dram_parameter("kp_o", [U * SKIP, J, 3], F32, isOutput=True)
    sk_o = nc.declare_dram_parameter("sk_o", [U * SKIP, J, 4, 4], F32, isOutput=True)
    rv_o = nc.declare_dram_parameter("rv_o", [U * SKIP, J, 3], F32, isOutput=True)

    with tile.TileContext(nc) as tc:
        with (
            tc.tile_pool(name="w", bufs=1) as wpool,
            tc.tile_pool(name="big", bufs=1) as big,
            tc.tile_pool(name="psum", bufs=4, space="PSUM") as psum,
            tc.tile_pool(name="psum4", bufs=4, space="PSUM") as psum4,
        ):
            # ---------------- inputs ----------------
            PACK = wpool.tile([P, PXC], F32)
            nc.sync.dma_start(PACK[:, 0:537], pk_i[:, 0:537])
            xT = big.tile([DIN, U], F32, tag="g_xt")       # slot reused by WQC
            nc.sync.dma_start(xT[:, 0:U // 2], xT_i[:, 0:U // 2])
            nc.sync.dma_start(xT[:, U // 2:], xT_i[:, U // 2:])
            nc.sync.dma_start(PACK[:, 537:], pk_i[:, 537:])
            STG = big.tile([P, DOUT * C], F32, tag="g_stg")
            nc.sync.dma_start(STG[:], st_i[:])
            w0 = PACK[0:DIN, 0:128]
            w1 = PACK[0:W, 128:256]
            w2 = PACK[0:W, 256:384]
            w3 = PACK[0:W, 384:459]
            b0 = PACK[0:W, 534:535]
            b1 = PACK[0:W, 535:536]
            b2 = PACK[0:W, 536:537]
            TL1 = PACK[:, 537:537 + 3 * JC]
            TL2 = PACK[:, 537 + 3 * JC:537 + 6 * JC]
            halfpi = wpool.tile([P, 1], F32)
            nc.vector.memset(halfpi[:], float(np.pi / 2))

            # skts constant row [0,0,0,1] written while engines are idle
            SK = big.tile([P, C * J * 16], F32, tag="g_skt")
            skv = SK.rearrange("p (c j e) -> p c j e", j=J, e=16)
            nc.vector.memset(skv[:, :, :, 12:15], 0.0)
            nc.vector.memset(skv[:, :, :, 15], 1.0)

            # ---------------- MLP ----------------
            NCHUNK = 512
            h1 = big.tile([W, U], F32, tag="g_h1")         # reused by RV2
            h2 = big.tile([W, U], F32, tag="g_h2")         # reused by PRC
            h3 = big.tile([W, U], F32, tag="g_h3")         # reused by WRC
            with nc.named_scope("mlp"):
                for m in range(U // NCHUNK):
                    sl = slice(m * NCHUNK, (m + 1) * NCHUNK)
                    ps = psum.tile([W, NCHUNK], F32, tag="mm")
                    nc.tensor.matmul(ps[:], mmcast(w0), mmcast(xT[:, sl]), start=True, stop=True)
                    nc.scalar.activation(h1[:, sl], ps[:], AF.Relu, bias=b0)
                for m in range(U // NCHUNK):
                    sl = slice(m * NCHUNK, (m + 1) * NCHUNK)
                    ps = psum.tile([W, NCHUNK], F32, tag="mm")
                    nc.tensor.matmul(ps[:], mmcast(w1), mmcast(h1[:, sl]), start=True, stop=True)
                    nc.scalar.activation(h2[:, sl], ps[:], AF.Relu, bias=b1)
                for m in range(U // NCHUNK):
                    sl = slice(m * NCHUNK, (m + 1) * NCHUNK)
                    ps = psum.tile([W, NCHUNK], F32, tag="mm")
                    nc.tensor.matmul(ps[:], mmcast(w2), mmcast(h2[:, sl]), start=True, stop=True)
                    nc.scalar.activation(h3[:, sl], ps[:], AF.Relu, bias=b2)
                # layer 4; residual-plus-base via fused STT (STG preloaded with base)
                for c in range(C):
                    ps = psum4.tile([P, DOUT], F32, tag="mm4")
                    nc.tensor.matmul(ps[:], mmcast(h3[:, c * P:(c + 1) * P]), mmcast(w3),
                                     start=True, stop=True)
                    sv = STG.rearrange("p (q c) -> p q c", c=C)[:, :, c]
                    nc.vector.scalar_tensor_tensor(sv, ps[:], 1.0, sv, ALU.mult, ALU.add)

            rx = STG[:, 0 * JC:1 * JC]
            ry = STG[:, 1 * JC:2 * JC]
            rz = STG[:, 2 * JC:3 * JC]

            # rvecs output depends only on STG: assemble + DMA it first
            RV2 = big.tile([P, C * 2 * J * 3], F32, tag="g_rv2")
            rvv = RV2.rearrange("p (c d j k) -> p c d j k", d=2, j=J, k=3)
            rv_cja = STG.rearrange("p (q c) -> p q c", c=C)[:, 0:J * 3, :] \
                        .rearrange("p (a j) c -> p c j a", a=3)
            with nc.named_scope("rv_out"):
                for d in range(2):
                    nc.scalar.copy(rvv[:, :, d], rv_cja)
                rv_flat = rv_o.rearrange("(c p r d) j k -> p c r (d j k)", c=C, p=P,
                                         r=SKIP // 2, d=2)
                rv_src = RV2.rearrange("p (c e) -> p c e", c=C)
                for r in range(SKIP // 2):
                    nc.sync.dma_start(rv_flat[:, :, r], rv_src[:])

            # ---------------- Rodrigues -> packed local-quat variants ----------------
            LQVC = big.tile([P, JC * 16], F32, tag="g_lqvc")
            lqv16 = LQVC.rearrange("p (g e) -> p g e", e=16)
            with nc.named_scope("rodrigues"):
                sqx = big.tile([P, JC], F32, tag="t0")
                sqy = big.tile([P, JC], F32, tag="t1")
                sqz = big.tile([P, JC], F32, tag="t2")
                nc.scalar.activation(sqx[:], rx, AF.Square)
                nc.scalar.activation(sqy[:], ry, AF.Square)
                nc.scalar.activation(sqz[:], rz, AF.Square)
                t2_ = big.tile([P, JC], F32, tag="t3")
                nc.vector.tensor_tensor(t2_[:], sqx[:], sqy[:], ALU.add)
                nc.vector.tensor_tensor(t2_[:], t2_[:], sqz[:], ALU.add)
                th = big.tile([P, JC], F32, tag="t4")
                nc.scalar.activation(th[:], t2_[:], AF.Sqrt)
                thc = big.tile([P, JC], F32, tag="t5")
                nc.vector.tensor_scalar_max(thc[:], th[:], 1e-8)
                inv = big.tile([P, JC], F32, tag="t6")
                scratch = big.tile([P, JC], F32, tag="t7")
                nc.vector.reciprocal_approx_accurate(inv[:], thc[:], scratch[:])
                sh = big.tile([P, JC], F32, tag="t8")
                nc.scalar.activation(sh[:], th[:], AF.Sin, scale=0.5)
                sv_ = big.tile([P, JC], F32, tag="t9")
                nc.vector.tensor_tensor(sv_[:], sh[:], inv[:], ALU.mult)
                nsv = big.tile([P, JC], F32, tag="t10")
                nc.vector.tensor_scalar_mul(nsv[:], sv_[:], -1.0)
                for v in range(4):
                    nc.scalar.activation(lqv16[:, :, v * 4 + v], th[:], AF.Sin,
                                         scale=0.5, bias=halfpi[:])
                rblk = {1: rx, 2: ry, 3: rz}
                for v, spec in enumerate(VAR_SPEC):
                    for pos, (src_, sgn) in enumerate(spec):
                        if src_ == 0:
                            continue
                        s_op = sv_ if sgn > 0 else nsv
                        nc.vector.tensor_tensor(lqv16[:, :, v * 4 + pos],
                                                rblk[src_], s_op[:], ALU.mult)

            # ---------------- quaternion chain ----------------
            WQI = big.tile([P, JC * 4], F32, tag="g_wqi")
            wqi4 = WQI.rearrange("p (g i) -> p g i", i=4)
            TMP = big.tile([P, 5 * C * 16], F32, tag="g_tmp")
            REP = big.tile([P, 3 * C * 4], F32, tag="g_rep")

            def chain_levels(levels):
                # per parent-run: TT (parent quat broadcast over variants) + reduce
                for lev in levels:
                    for (clo, chi, plo, bc) in PRUNS[lev]:
                        w_ = chi - clo
                        if bc:
                            # replicate the single parent block w_ times first
                            rsrc = WQI[:, plo * C * 4:(plo + 1) * C * 4] \
                                .rearrange("p (j g) -> p j g", j=1) \
                                .broadcast_to([P, w_, C * 4])
                            rep = REP[:, 0:w_ * C * 4]
                            nc.vector.tensor_copy(rep.rearrange("p (j g) -> p j g", g=C * 4), rsrc)
                            par = rep
                        else:
                            par = WQI[:, plo * C * 4:(plo + w_) * C * 4]
                        a_in = par.rearrange("p (g v i) -> p g v i", v=1, i=4) \
                                  .broadcast_to([P, w_ * C, 4, 4])
                        b_in = LQVC[:, clo * C * 16:chi * C * 16] \
                            .rearrange("p (g v i) -> p g v i", v=4, i=4)
                        tview = TMP[:, 0:w_ * C * 16].rearrange("p (g v i) -> p g v i", v=4, i=4)
                        nc.vector.tensor_tensor(tview, a_in, b_in, ALU.mult)
                        out = WQI[:, clo * C * 4:chi * C * 4].rearrange("p (g v) -> p g v", v=4)
                        nc.vector.tensor_reduce(out, tview, axis=AX.X, op=ALU.add)

            with nc.named_scope("chain_root"):
                # root world quat = root local quat
                nc.scalar.activation(wqi4[:, 0:C, 0], th[:, 0:C], AF.Sin,
                                     scale=0.5, bias=halfpi[:])
                for i, blk in ((1, rx), (2, ry), (3, rz)):
                    nc.vector.tensor_tensor(wqi4[:, 0:C, i], blk[:, 0:C],
                                            sv_[:, 0:C], ALU.mult)

            # ---------------- per-group post-chain tiles ----------------
            WQC = big.tile([P, JC * 4], F32, tag="g_xt")
            WRC = big.tile([P, JC * 9], F32, tag="g_h3")
            wrb = [WRC[:, m * JC:(m + 1) * JC] for m in range(9)]
            PRC = big.tile([P, JC * 9], F32, tag="g_h2")
            dC = big.tile([P, JC * 3], F32, tag="g_dc")
            WTC = big.tile([P, JC * 3], F32, tag="g_wtc")
            TII = big.tile([P, JC * 3], F32, tag="g_tii")
            KP2 = big.tile([P, C * 2 * J * 3], F32, tag="g_bt")
            kpv = KP2.rearrange("p (c d j k) -> p c d j k", d=2, j=J, k=3)

            qw = WQC[:, 0 * JC:1 * JC]
            qx = WQC[:, 1 * JC:2 * JC]
            qy = WQC[:, 2 * JC:3 * JC]
            qz = WQC[:, 3 * JC:4 * JC]
            wr_mjc = WRC.rearrange("p (m j c) -> p m j c", m=9, j=J)
            pr_mjc = PRC.rearrange("p (m j c) -> p m j c", m=9, j=J)
            prb = [PRC[:, m * JC:(m + 1) * JC] for m in range(9)]
            tl1b = [TL1[:, k * JC:(k + 1) * JC] for k in range(3)]
            tl2b = [TL2[:, k * JC:(k + 1) * JC] for k in range(3)]
            wt_kjc = WTC.rearrange("p (k j c) -> p k j c", k=3, j=J)
            dc_kjc = dC.rearrange("p (k j c) -> p k j c", k=3, j=J)
            wtb = [WTC[:, k * JC:(k + 1) * JC] for k in range(3)]
            wr_cj = WRC.rearrange("p (m j c) -> p m c j", m=9, j=J)
            ti_cj = TII.rearrange("p (r j c) -> p r c j", r=3, j=J)
            wt_cjk = WTC.rearrange("p (k j c) -> p c j k", k=3, j=J)
            sk_flat = sk_o.rearrange("(c p r) j a b -> p c r j (a b)", c=C, p=P, r=SKIP)
            sk_srcj = SK.rearrange("p (c j e) -> p c j e", j=J, e=16)

            def sl_(blk, lo, hi):
                # jc-slice [lo*C, hi*C) of a 384-wide comp block view
                return blk[:, lo * C:hi * C]

            def emit_group(g):
                jlo, jhi = g
                S = slice(jlo * C, jhi * C)
                with nc.named_scope(f"deint_{jlo}"):
                    for i in range(4):
                        nc.scalar.copy(WQC[:, i * JC + jlo * C:i * JC + jhi * C],
                                       wqi4[:, S, i])
                with nc.named_scope(f"q2m_{jlo}"):
                    xx = big.tile([P, JC], F32, tag="t0")
                    yy = big.tile([P, JC], F32, tag="t1")
                    zz = big.tile([P, JC], F32, tag="t2")
                    xy = big.tile([P, JC], F32, tag="t3")
                    xz = big.tile([P, JC], F32, tag="t4")
                    yz = big.tile([P, JC], F32, tag="t5")
                    wx = big.tile([P, JC], F32, tag="t6")
                    wy = big.tile([P, JC], F32, tag="t7")
                    wz = big.tile([P, JC], F32, tag="t8")
                    s_ = big.tile([P, JC], F32, tag="t9")
                    for dst_, a_ in ((xx, qx), (yy, qy), (zz, qz)):
                        nc.scalar.activation(sl_(dst_, jlo, jhi), sl_(a_, jlo, jhi), AF.Square)
                    for dst_, a_, b_ in ((xy, qx, qy), (xz, qx, qz), (yz, qy, qz),
                                         (wx, qw, qx), (wy, qw, qy), (wz, qw, qz)):
                        nc.vector.tensor_tensor(sl_(dst_, jlo, jhi), sl_(a_, jlo, jhi),
                                                sl_(b_, jlo, jhi), ALU.mult)
                    for m, p1, p2 in ((0, yy, zz), (4, xx, zz), (8, xx, yy)):
                        nc.vector.tensor_tensor(sl_(s_, jlo, jhi), sl_(p1, jlo, jhi),
                                                sl_(p2, jlo, jhi), ALU.add)
                        nc.scalar.activation(sl_(wrb[m], jlo, jhi), sl_(s_, jlo, jhi),
                                             AF.Copy, scale=-2.0, bias=1.0)
                    for m, p1, p2, op in ((1, xy, wz, ALU.add), (3, xy, wz, ALU.subtract),
                                          (2, xz, wy, ALU.subtract), (6, xz, wy, ALU.add),
                                          (5, yz, wx, ALU.add), (7, yz, wx, ALU.subtract)):
                        nc.vector.tensor_tensor(sl_(wrb[m], jlo, jhi), sl_(p1, jlo, jhi),
                                                sl_(p2, jlo, jhi), op)
                with nc.named_scope(f"deltas_{jlo}"):
                    if jlo == 0:
                        nc.vector.memset(pr_mjc[:, :, 0], 0.0)
                    for (clo, chi, plo, bc) in G_TREE[g]:
                        dst = pr_mjc[:, :, clo:chi]
                        if bc:
                            src = wr_mjc[:, :, plo:plo + 1].broadcast_to([P, 9, chi - clo, C])
                        else:
                            src = wr_mjc[:, :, plo:plo + (chi - clo)]
                        nc.scalar.copy(dst, src)
                    m1 = big.tile([P, JC], F32, tag="t0")
                    m2 = big.tile([P, JC], F32, tag="t1")
                    for a in range(3):
                        acc = sl_(dC[:, a * JC:(a + 1) * JC], jlo, jhi)
                        tls = [tl1b[k] if k == a else tl2b[k] for k in range(3)]
                        nc.vector.tensor_tensor(sl_(m1, jlo, jhi), sl_(prb[0 * 3 + a], jlo, jhi),
                                                sl_(tls[0], jlo, jhi), ALU.mult)
                        nc.vector.tensor_tensor(sl_(m2, jlo, jhi), sl_(prb[1 * 3 + a], jlo, jhi),
                                                sl_(tls[1], jlo, jhi), ALU.mult)
                        nc.vector.tensor_tensor(sl_(m1, jlo, jhi), sl_(m1, jlo, jhi),
                                                sl_(m2, jlo, jhi), ALU.add)
                        nc.vector.tensor_tensor(sl_(m2, jlo, jhi), sl_(prb[2 * 3 + a], jlo, jhi),
                                                sl_(tls[2], jlo, jhi), ALU.mult)
                        nc.vector.tensor_tensor(acc, sl_(m1, jlo, jhi), sl_(m2, jlo, jhi),
                                                ALU.add)
                with nc.named_scope(f"prefix_{jlo}"):
                    if jlo == 0:
                        pel = STG.rearrange("p (q c) -> p q c", c=C)[:, DOUT - 3:DOUT, :]
                        nc.vector.tensor_copy(wt_kjc[:, :, 0], pel)
                    for (clo, chi, plo, bc) in G_PREFIX[g]:
                        dst = wt_kjc[:, :, clo:chi]
                        if bc:
                            src = wt_kjc[:, :, plo:plo + 1].broadcast_to([P, 3, chi - clo, C])
                        else:
                            src = wt_kjc[:, :, plo:plo + (chi - clo)]
                        nc.vector.tensor_tensor(dst, src, dc_kjc[:, :, clo:chi], ALU.add)
                if jlo == GLAST[0]:
                    with nc.named_scope("kp_out"):
                        for d_ in range(2):
                            nc.scalar.copy(kpv[:, :, d_], wt_cjk[:])
                        kp_flat = kp_o.rearrange("(c p r d) j k -> p c r (d j k)", c=C,
                                                 p=P, r=SKIP // 2, d=2)
                        kp_src = KP2.rearrange("p (c e) -> p c e", c=C)
                        for r_ in range(SKIP // 2):
                            nc.sync.dma_start(kp_flat[:, :, r_], kp_src[:])
                with nc.named_scope(f"tinv_{jlo}"):
                    n1 = big.tile([P, JC], F32, tag="t2")
                    n2 = big.tile([P, JC], F32, tag="t3")
                    for r in range(3):
                        acc = sl_(TII[:, r * JC:(r + 1) * JC], jlo, jhi)

                        def term(dst_, k, r=r):
                            if k == r:
                                nc.vector.tensor_tensor(dst_, sl_(wrb[r * 3 + k], jlo, jhi),
                                                        sl_(wtb[k], jlo, jhi), ALU.mult)
                            else:
                                nc.vector.scalar_tensor_tensor(dst_, sl_(wrb[r * 3 + k], jlo, jhi),
                                                               2.0, sl_(wtb[k], jlo, jhi),
                                                               ALU.mult, ALU.mult)
                        term(sl_(n1, jlo, jhi), 0)
                        term(sl_(n2, jlo, jhi), 1)
                        nc.vector.tensor_tensor(sl_(n1, jlo, jhi), sl_(n1, jlo, jhi),
                                                sl_(n2, jlo, jhi), ALU.add)
                        term(sl_(n2, jlo, jhi), 2)
                        nc.vector.tensor_tensor(acc, sl_(n1, jlo, jhi), sl_(n2, jlo, jhi),
                                                ALU.add)
                with nc.named_scope(f"sk_{jlo}"):
                    for r in range(3):
                        for k in range(3):
                            scale = 1.0 if r == k else 2.0
                            if r == 1:
                                nc.vector.tensor_scalar_mul(skv[:, :, jlo:jhi, r * 4 + k],
                                                            wr_cj[:, r * 3 + k, :, jlo:jhi],
                                                            scale)
                            else:
                                nc.scalar.activation(skv[:, :, jlo:jhi, r * 4 + k],
                                                     wr_cj[:, r * 3 + k, :, jlo:jhi],
                                                     AF.Copy, scale=scale)
                        if r == 1:
                            nc.vector.tensor_scalar_mul(skv[:, :, jlo:jhi, r * 4 + 3],
                                                        ti_cj[:, r, :, jlo:jhi], -1.0)
                        else:
                            nc.scalar.activation(skv[:, :, jlo:jhi, r * 4 + 3],
                                                 ti_cj[:, r, :, jlo:jhi], AF.Copy, scale=-1.0)
                with nc.named_scope(f"skdma_{jlo}"):
                    eng = nc.sync if G_DMA_ENG[g] == "sync" else nc.scalar
                    for r in range(SKIP):
                        eng.dma_start(sk_flat[:, :, r, jlo:jhi],
                                      sk_srcj[:, :, jlo:jhi])

            # chain piece -> group, interleaved for earliest first output
            for g in GROUPS:
                with nc.named_scope(f"chain_{g[0]}"):
                    chain_levels(G_CHAIN[g])
                emit_group(g)

            if debug_taps:
                nc.sync.dma_start(dbg["lqvc"][:], LQVC[:])
                nc.sync.dma_start(dbg["wqi"][:], WQI[:])
                nc.sync.dma_start(dbg["wrc"][:], WRC[:])
                nc.sync.dma_start(dbg["dc"][:], dC[:])
                nc.sync.dma_start(dbg["wtc"][:], WTC[:])
                nc.sync.dma_start(dbg["tii"][:], TII[:])

    nc.compile()
    return nc


def _get_nc():
    if "nc" not in _CACHE:
        _CACHE["nc"] = _build()
    return _CACHE["nc"]


def _maybe_enable_trace():
    import types
    try:
        import antenv
        if hasattr(antenv, "axon_hooks"):
            return True
        hooks_mod = types.ModuleType("antenv.axon_hooks")
        _hook = [None]
        hooks_mod.set_axon_ntff_profile_hook = lambda h: _hook.__setitem__(0, h)
        hooks_mod.get_axon_ntff_profile_hook = lambda: _hook[0]
        sys.modules["antenv.axon_hooks"] = hooks_mod
        import antenv as _ae
        _ae.axon_hooks = hooks_mod
        from trn_agent_boot.trn_boot import _ntff_profile_via_ctypes
        hooks_mod.set_axon_ntff_profile_hook(
            _ntff_profile_via_ctypes("/opt/axon/libaxon_pjrt.so"))
        return True
    except Exception:
        return False


def kernel(bones, kp3d, kp_idxs, emb_table, pelvis_buf, rest_pose,
           W0, b0, W1, b1, W2, b2, W3, b3, N_unique):
    from concourse.bass_utils import run_bass_kernel_spmd

    bones = np.asarray(bones, dtype=np.float32)
    emb_table = np.asarray(emb_table, dtype=np.float32)
    pelvis_buf = np.asarray(pelvis_buf, dtype=np.float32)
    rest_pose = np.asarray(rest_pose, dtype=np.float32)
    kp_idxs = np.asarray(kp_idxs)
    W0 = np.asarray(W0, dtype=np.float32); b0 = np.asarray(b0, dtype=np.float32)
    W1 = np.asarray(W1, dtype=np.float32); b1 = np.asarray(b1, dtype=np.float32)
    W2 = np.asarray(W2, dtype=np.float32); b2 = np.asarray(b2, dtype=np.float32)
    W3 = np.asarray(W3, dtype=np.float32); b3 = np.asarray(b3, dtype=np.float32)

    nu = int(N_unique)
    assert nu == NU and bones.shape[0] == N

    rvecs = bones[::SKIP].reshape(nu, J * 3)
    kp_u = np.asarray(kp_idxs[::SKIP], dtype=np.int64)
    embs = emb_table[kp_u]
    pelvis = pelvis_buf[kp_u]

    qperm = np.empty(DOUT, np.int64)
    for q in range(72):
        qperm[q] = (q % J) * 3 + (q // J)
    qperm[72:] = np.arange(72, 75)
    W3p = np.ascontiguousarray(W3[:, qperm] * RES_SCALE)
    b3p = (b3[qperm] * RES_SCALE).astype(np.float32)

    tl = np.zeros((J, 3), np.float32)
    for j in range(1, J):
        tl[j] = rest_pose[j] - rest_pose[PARENTS[j]]
    # comp-major TL rows: [k*JC + j*16 + c] = tl[j, k] (and 2x version)
    tlc = np.repeat(tl.T.reshape(3, J), C, axis=1).reshape(3 * JC)
    pack = np.zeros((P, 2841), np.float32)
    pack[0:DIN, 0:128] = W0
    pack[0:W, 128:256] = W1
    pack[0:W, 256:384] = W2
    pack[0:W, 384:459] = W3p
    pack[0:DOUT, 459:534] = np.eye(DOUT, dtype=np.float32)
    pack[0:W, 534] = b0.ravel()
    pack[0:W, 535] = b1.ravel()
    pack[0:W, 536] = b2.ravel()
    pack[:, 537:537 + 3 * JC] = tlc
    pack[:, 537 + 3 * JC:537 + 6 * JC] = 2.0 * tlc

    in_maps = []
    for core in range(NCORES):
        sl = slice(core * U, (core + 1) * U)
        rv_c = rvecs[sl]
        x = np.concatenate([rv_c, embs[sl]], axis=1)
        xT = np.ascontiguousarray(x.T)
        base = np.empty((U, DOUT), np.float32)
        base[:, :72] = rv_c.reshape(U, J, 3).transpose(0, 2, 1).reshape(U, 72)
        base[:, 72:] = pelvis[sl]
        base += b3p
        # stg0[p, q*16+c] = base[c*128+p, q]
        stg0 = np.ascontiguousarray(
            base.reshape(C, P, DOUT).transpose(1, 2, 0).reshape(P, DOUT * C))
        in_maps.append(dict(xT=xT, stg0=stg0, pack=pack))

    trace = bool(os.environ.get("KERNEL_TRACE")) and _maybe_enable_trace()
    nc = _get_nc()
    res = run_bass_kernel_spmd(nc, in_maps, core_ids=list(range(NCORES)), trace=trace)
    if trace:
        _CACHE["last_exec_ns"] = res.exec_time_ns
        _CACHE["last_trace"] = res.instructions_and_trace[1] if res.instructions_and_trace else None
        _CACHE["last_scope_times"] = res.per_core_scope_times

    _CACHE["last_res"] = res.results
    kp = np.concatenate([r["kp_o"] for r in res.results], axis=0)
    skts = np.concatenate([r["sk_o"] for r in res.results], axis=0)
    rv = np.concatenate([r["rv_o"] for r in res.results], axis=0)
    return kp, skts, rv
